# revision 28
# baseline (speedup 1.0000x reference)
"""Trainium2 Bass kernel for nn_BinarizedLinear:
    out = sign(input_b @ sign(weight).T)
with input_b (8192, 2048) and weight (2048, 2048), entries all +/-1.0 fp32.

Since weight entries are +/-1, sign(weight) == weight; the linear output is a
sum of 2048 +/-1 terms, i.e. an even integer in [-2048, 2048], so
sign(v) == clamp(v, -1, 1) exactly, and bf16/fp8 operands are bit-exact
(+/-1 is representable; PSUM accumulates in fp32).

Strategy: data-parallel across 8 NeuronCores — each core gets 1024 rows of
input_b, the full weight replicated.  Per core:
  - DMA fp32 slabs in (half-slab granularity for latency), cast to bf16 on
    DVE (dedicated engine so casts never queue behind evictions),
  - transpose x and W tiles on the TensorEngine (the matmul contracts along
    SBUF partitions, so both operands need the k dim partition-major);
    8 transposes pack into one PSUM bank so each ACT eviction moves 1024
    columns, casting bf16 -> fp8e4 in the same copy,
  - fp8 matmuls with perf_mode=DoubleRow (2 k-tiles per pass, 2x PE rate),
    accumulating k=2048 into PSUM fp32 — exact since products are +/-1,
  - fuse sign() into the PSUM->SBUF eviction as a single DVE tensor_scalar
    (min 1.0 then max -1.0), DMA the fp32 result out on the GpSimd SWDGE
    queue (keeps the sync HWDGE queue free of head-of-line blocking).
Scheduling: transposed W is only 4MB at fp8 so it is cached in SBUF whole;
x/W slab production is spread evenly across the kernel, interleaved ~1:1
with matmul blocks, and matmul blocks are emitted with one-slab-lag
availability so they never wait on the eviction of the slab just emitted.
A dummy-matmul warmup burst during the DMA-bound startup flips the PE's HAM
clock gate to full rate before real work lands.
Measured on trn2 (8 cores, NTFF profile): ~120-128 us, bit-exact output.
"""

import numpy as np

BATCH, IN_LEN, OUT_LEN = 8192, 2048, 2048
N_CORES = 8
SHARD = BATCH // N_CORES  # 1024
P = 128

_cache = {}


def build_kernel(shard=SHARD, in_len=IN_LEN, out_len=OUT_LEN,
                 use_double_row=True):
    import concourse.mybir as mybir
    import concourse.tile as tile
    from concourse import bacc
    from concourse.masks import make_identity

    f32 = mybir.dt.float32
    bf16 = mybir.dt.bfloat16
    fp8 = mybir.dt.float8e4

    mm_dt = fp8 if use_double_row else bf16

    KT = in_len // P          # k-tiles (contraction)
    BT = shard // P           # b-tiles per core
    OB = out_len // 512       # 512-wide output blocks
    OJ = 512 // P             # 128-row W chunks per o-block
    KP = min(8, KT)           # transposes packed per PSUM bank eviction

    nc = bacc.Bacc(None, target_bir_lowering=False)
    x = nc.dram_tensor("x", [shard, in_len], f32, kind="ExternalInput")
    w = nc.dram_tensor("w", [out_len, in_len], f32, kind="ExternalInput")
    out = nc.dram_tensor("out", [shard, out_len], f32, kind="ExternalOutput")
    scratch = nc.dram_tensor("scratch", [1, 1], f32, kind="ExternalOutput")

    with tile.TileContext(nc) as tc:
        with (
            tc.tile_pool(name="const", bufs=1) as const_pool,
            tc.tile_pool(name="xt", bufs=BT) as xt_pool,
            tc.tile_pool(name="stage", bufs=10) as stage_pool,
            tc.tile_pool(name="bstage", bufs=8) as bstage_pool,
            tc.tile_pool(name="wtblk", bufs=out_len // 512) as wt_pool,
            tc.tile_pool(name="outs", bufs=6) as out_pool,
            tc.tile_pool(name="tpsum", bufs=3, space="PSUM") as tpsum_pool,
            tc.tile_pool(name="mpsum", bufs=4, space="PSUM") as mpsum_pool,
            tc.tile_pool(name="wpsum", bufs=1, space="PSUM") as wpsum_pool,
        ):
            # HAM warmup: the PE is otherwise idle for the first ~12us
            # (framework preamble + first DMA+cast); a burst of dummy
            # matmuls flips the PE clock gate to 8/8 before real work lands,
            # which otherwise runs at half clock for the first ~20us.
            WARM = 32
            warm_src = const_pool.tile([P, 512], bf16, name="warm_src")
            nc.gpsimd.memset(warm_src[:], 1.0)
            warm_psum = wpsum_pool.tile([P, 512], f32, name="warm_psum")
            for i in range(WARM):
                nc.tensor.matmul(
                    warm_psum[:], warm_src[:, :P], warm_src[:],
                    start=(i == 0), stop=(i == WARM - 1),
                )
            warm_out = const_pool.tile([1, 1], f32, name="warm_out")
            nc.vector.tensor_copy(out=warm_out[:], in_=warm_psum[:1, :1])
            nc.gpsimd.dma_start(out=scratch[:], in_=warm_out[:])

            ident = const_pool.tile([P, P], bf16)
            make_identity(nc, ident)

            HALF = in_len // 2

            def load_cast(dram, row0):
                """DMA a [128, in_len] fp32 slab in two halves, cast each to
                bf16 on DVE as soon as it lands (finer pipelining; casts get
                a dedicated engine so they never queue behind evictions)."""
                halves = []
                for h in range(2):
                    stage = stage_pool.tile([P, HALF], f32, tag="stage")
                    nc.sync.dma_start(
                        out=stage[:],
                        in_=dram[row0:row0 + P, h * HALF:(h + 1) * HALF],
                    )
                    bst = bstage_pool.tile([P, HALF], bf16, tag="bstage")
                    nc.vector.tensor_copy(out=bst[:], in_=stage[:])
                    halves.append(bst)
                return halves

            def transpose_into(halves, dest_fn):
                """PE-transpose [128,128] sub-tiles; pack KP per PSUM bank,
                then evict each bank with one wide copy (on ACT) via dest_fn."""
                for k0 in range(0, KT, KP):
                    tp = tpsum_pool.tile([P, KP * P], bf16, tag="tp")
                    for q in range(KP):
                        col = (k0 + q) * P
                        bst = halves[col // HALF]
                        col -= (col // HALF) * HALF
                        nc.tensor.transpose(
                            tp[:, q * P:(q + 1) * P],
                            bst[:, col:col + P],
                            ident[:],
                        )
                    dest_fn(k0, tp)

            # xt[bt][p, k, b] = x[bt*128 + b, k*128 + p]
            xt = {}

            def emit_x(bt):
                halves = load_cast(x, bt * P)
                xt[bt] = xt_pool.tile([P, KT, P], mm_dt, tag="xt",
                                      name=f"xt{bt}")

                def dest(k0, tp):
                    nc.scalar.copy(
                        out=xt[bt][:, k0:k0 + KP, :],
                        in_=tp[:].rearrange("p (k b) -> p k b", k=KP),
                    )

                transpose_into(halves, dest)

            # wt_blk[p, k, j*128+o] = w[ob*512 + j*128 + o, k*128 + p]
            def emit_w_chunk(wt_blk, ob, j):
                halves = load_cast(w, (ob * OJ + j) * P)

                def dest(k0, tp):
                    nc.scalar.copy(
                        out=wt_blk[:, k0:k0 + KP, j * P:(j + 1) * P],
                        in_=tp[:].rearrange("p (k b) -> p k b", k=KP),
                    )

                transpose_into(halves, dest)

            def emit_mm(wt_blk, ob, bt, out_eng=None):
                psum = mpsum_pool.tile([P, 512], f32)
                if use_double_row:
                    for q in range(KT // 2):
                        nc.tensor.matmul(
                            psum[:],
                            xt[bt][:, 2 * q:2 * q + 2, :],
                            wt_blk[:, 2 * q:2 * q + 2, :],
                            start=(q == 0),
                            stop=(q == KT // 2 - 1),
                            perf_mode=mybir.MatmulPerfMode.DoubleRow,
                        )
                else:
                    for k in range(KT):
                        nc.tensor.matmul(
                            psum[:],
                            xt[bt][:, k, :],
                            wt_blk[:, k, :],
                            start=(k == 0),
                            stop=(k == KT - 1),
                        )
                ot = out_pool.tile([P, 512], f32)
                # sign(v) for integer v: clamp to [-1, 1]
                nc.vector.tensor_scalar(
                    out=ot[:], in0=psum[:], scalar1=1.0, scalar2=-1.0,
                    op0=mybir.AluOpType.min, op1=mybir.AluOpType.max,
                )
                (out_eng or nc.gpsimd).dma_start(
                    out=out[bt * P:(bt + 1) * P, ob * 512:(ob + 1) * 512],
                    in_=ot[:],
                )

            # All of transposed W fits in SBUF at fp8 (32KB/partition), so
            # production (DMA+cast+transpose+evict) of x and W slabs is
            # spread evenly across the whole kernel, interleaved ~1:1 with
            # matmul blocks; engines stay balanced instead of front-loading
            # all x work into o-block 0.
            wt_blks = {
                ob: wt_pool.tile([P, KT, 512], mm_dt, tag="wtblk",
                                 name=f"wt{ob}")
                for ob in range(OB)
            }

            # Order: 2 x slabs + all of W block 0 up front (the minimum for
            # the first matmul block), then alternate one x slab per W chunk
            # so newly-unlocked matmul blocks never run dry — a multi-slab
            # stretch of pure transposes lets the HAM clock gate re-throttle
            # the PE to half clock.
            production = []
            x_left = list(range(BT))
            n_lead = min(2, BT)
            for bt in x_left[:n_lead]:
                production.append(("x", bt))
            x_left = x_left[n_lead:]
            w_items = [("w", ob, j) for ob in range(OB) for j in range(OJ)]
            production.extend(w_items[:OJ])
            for it in w_items[OJ:]:
                if x_left:
                    production.append(("x", x_left.pop(0)))
                production.append(it)
            for bt in x_left:
                production.append(("x", bt))

            x_done, w_done = set(), set()
            mm_todo = [(ob, bt) for ob in range(OB) for bt in range(BT)]

            def flush_mms(limit, xs, ws, out_eng=None):
                n = 0
                for item in list(mm_todo):
                    ob, bt = item
                    if ob in ws and bt in xs and n < limit:
                        emit_mm(wt_blks[ob], ob, bt, out_eng)
                        mm_todo.remove(item)
                        n += 1

            # flush with one-production-lag availability so a matmul block
            # never waits on the eviction of the slab emitted right before it
            prev_x, prev_w = set(), set()
            n_produced = 0
            for item in production:
                # short, fully-closed dummy-matmul groups during the
                # transpose-heavy ramp: HAM ignores transpose-mode activity,
                # so without matmul traffic the PE drops to half clock until
                # real matmul blocks start flowing (~6 slabs in)
                if 0 < n_produced < 7:
                    for i in range(4):
                        nc.tensor.matmul(
                            warm_psum[:], warm_src[:, :P], warm_src[:],
                            start=(i == 0), stop=(i == 3),
                        )
                n_produced += 1
                if item[0] == "x":
                    emit_x(item[1])
                    x_done.add(item[1])
                else:
                    _, ob, j = item
                    emit_w_chunk(wt_blks[ob], ob, j)
                    if j == OJ - 1:
                        w_done.add(ob)
                flush_mms(2, prev_x, prev_w)
                prev_x, prev_w = set(x_done), set(w_done)
            flush_mms(len(mm_todo), x_done, w_done, out_eng=nc.sync)

    nc.finalize()
    return nc


def _get_nc():
    if "nc" not in _cache:
        _cache["nc"] = build_kernel()
    return _cache["nc"]


def run_sharded(input_b, weight, trace=False):
    """Run the SPMD kernel; returns (output, BassKernelResults)."""
    from concourse.bass_utils import run_bass_kernel_spmd

    nc = _get_nc()
    input_b = np.ascontiguousarray(input_b, dtype=np.float32)
    weight = np.ascontiguousarray(weight, dtype=np.float32)
    in_maps = [
        {"x": input_b[c * SHARD:(c + 1) * SHARD], "w": weight}
        for c in range(N_CORES)
    ]
    res = run_bass_kernel_spmd(nc, in_maps, list(range(N_CORES)), trace=trace)
    out = np.concatenate([res.results[c]["out"] for c in range(N_CORES)], axis=0)
    return out, res


def kernel(input_b, weight):
    out, _ = run_sharded(input_b, weight, trace=False)
    return out


# revision 29
# speedup vs baseline: 1.0336x; 1.0336x over previous
"""Trainium2 Bass kernel for nn_BinarizedLinear:
    out = sign(input_b @ sign(weight).T)
with input_b (8192, 2048) and weight (2048, 2048), entries all +/-1.0 fp32.

Since weight entries are +/-1, sign(weight) == weight; the linear output is a
sum of 2048 +/-1 terms, i.e. an even integer in [-2048, 2048], so
sign(v) == clamp(v, -1, 1) exactly, and bf16/fp8 operands are bit-exact
(+/-1 is representable; PSUM accumulates in fp32).

Strategy: data-parallel across 8 NeuronCores — each core gets 1024 rows of
input_b, the full weight replicated.  Per core:
  - DMA fp32 slabs in (half-slab granularity for latency), cast to bf16 on
    DVE (dedicated engine so casts never queue behind evictions),
  - transpose x and W tiles on the TensorEngine (the matmul contracts along
    SBUF partitions, so both operands need the k dim partition-major);
    8 transposes pack into one PSUM bank so each ACT eviction moves 1024
    columns, casting bf16 -> fp8e4 in the same copy,
  - fp8 matmuls with perf_mode=DoubleRow (2 k-tiles per pass, 2x PE rate),
    accumulating k=2048 into PSUM fp32 — exact since products are +/-1,
  - fuse sign() into the PSUM->SBUF eviction as a single DVE tensor_scalar
    (min 1.0 then max -1.0), DMA the fp32 result out on the GpSimd SWDGE
    queue (keeps the sync HWDGE queue free of head-of-line blocking).
Scheduling: transposed W is only 4MB at fp8 so it is cached in SBUF whole;
x/W slab production is spread evenly across the kernel, interleaved ~1:1
with matmul blocks, and matmul blocks are emitted with one-slab-lag
availability so they never wait on the eviction of the slab just emitted.
A dummy-matmul warmup burst during the DMA-bound startup flips the PE's HAM
clock gate to full rate before real work lands.
Measured on trn2 (8 cores, NTFF profile): ~120-128 us, bit-exact output.
"""

import numpy as np

BATCH, IN_LEN, OUT_LEN = 8192, 2048, 2048
N_CORES = 8
SHARD = BATCH // N_CORES  # 1024
P = 128

_cache = {}


def build_kernel(shard=SHARD, in_len=IN_LEN, out_len=OUT_LEN,
                 use_double_row=True):
    import concourse.mybir as mybir
    import concourse.tile as tile
    from concourse import bacc
    from concourse.masks import make_identity

    f32 = mybir.dt.float32
    bf16 = mybir.dt.bfloat16
    fp8 = mybir.dt.float8e4

    mm_dt = fp8 if use_double_row else bf16

    KT = in_len // P          # k-tiles (contraction)
    BT = shard // P           # b-tiles per core
    OB = out_len // 512       # 512-wide output blocks
    OJ = 512 // P             # 128-row W chunks per o-block
    KP = min(8, KT)           # transposes packed per PSUM bank eviction

    nc = bacc.Bacc(None, target_bir_lowering=False)
    x = nc.dram_tensor("x", [shard, in_len], f32, kind="ExternalInput")
    w = nc.dram_tensor("w", [out_len, in_len], f32, kind="ExternalInput")
    out = nc.dram_tensor("out", [shard, out_len], f32, kind="ExternalOutput")
    scratch = nc.dram_tensor("scratch", [1, 1], f32, kind="ExternalOutput")

    with tile.TileContext(nc) as tc:
        with (
            tc.tile_pool(name="const", bufs=1) as const_pool,
            tc.tile_pool(name="xt", bufs=BT) as xt_pool,
            tc.tile_pool(name="stage", bufs=10) as stage_pool,
            tc.tile_pool(name="bstage", bufs=8) as bstage_pool,
            tc.tile_pool(name="wtblk", bufs=out_len // 512) as wt_pool,
            tc.tile_pool(name="outs", bufs=6) as out_pool,
            tc.tile_pool(name="tpsum", bufs=3, space="PSUM") as tpsum_pool,
            tc.tile_pool(name="mpsum", bufs=4, space="PSUM") as mpsum_pool,
            tc.tile_pool(name="wpsum", bufs=1, space="PSUM") as wpsum_pool,
        ):
            # HAM warmup: the PE is otherwise idle for the first ~12us
            # (framework preamble + first DMA+cast); a burst of dummy
            # matmuls flips the PE clock gate to 8/8 before real work lands,
            # which otherwise runs at half clock for the first ~20us.
            WARM = 32
            warm_src = const_pool.tile([P, 512], bf16, name="warm_src")
            nc.gpsimd.memset(warm_src[:], 1.0)
            warm_psum = wpsum_pool.tile([P, 512], f32, name="warm_psum")
            for i in range(WARM):
                nc.tensor.matmul(
                    warm_psum[:], warm_src[:, :P], warm_src[:],
                    start=(i == 0), stop=(i == WARM - 1),
                )
            warm_out = const_pool.tile([1, 1], f32, name="warm_out")
            nc.vector.tensor_copy(out=warm_out[:], in_=warm_psum[:1, :1])
            nc.gpsimd.dma_start(out=scratch[:], in_=warm_out[:])

            ident = const_pool.tile([P, P], bf16)
            make_identity(nc, ident)

            HALF = in_len // 2

            def load_cast(dram, row0):
                """DMA a [128, in_len] fp32 slab in two halves, cast each to
                bf16 on DVE as soon as it lands (finer pipelining; casts get
                a dedicated engine so they never queue behind evictions)."""
                halves = []
                for h in range(2):
                    stage = stage_pool.tile([P, HALF], f32, tag="stage")
                    nc.sync.dma_start(
                        out=stage[:],
                        in_=dram[row0:row0 + P, h * HALF:(h + 1) * HALF],
                    )
                    bst = bstage_pool.tile([P, HALF], bf16, tag="bstage")
                    nc.vector.tensor_copy(out=bst[:], in_=stage[:])
                    halves.append(bst)
                return halves

            def transpose_into(halves, dest_fn):
                """PE-transpose [128,128] sub-tiles; pack KP per PSUM bank,
                then evict each bank with one wide copy (on ACT) via dest_fn."""
                for k0 in range(0, KT, KP):
                    tp = tpsum_pool.tile([P, KP * P], bf16, tag="tp")
                    for q in range(KP):
                        col = (k0 + q) * P
                        bst = halves[col // HALF]
                        col -= (col // HALF) * HALF
                        nc.tensor.transpose(
                            tp[:, q * P:(q + 1) * P],
                            bst[:, col:col + P],
                            ident[:],
                        )
                    dest_fn(k0, tp)

            # xt[bt][p, k, b] = x[bt*128 + b, k*128 + p]
            xt = {}

            def emit_x(bt):
                halves = load_cast(x, bt * P)
                xt[bt] = xt_pool.tile([P, KT, P], mm_dt, tag="xt",
                                      name=f"xt{bt}")

                def dest(k0, tp):
                    nc.scalar.copy(
                        out=xt[bt][:, k0:k0 + KP, :],
                        in_=tp[:].rearrange("p (k b) -> p k b", k=KP),
                    )

                transpose_into(halves, dest)

            # wt_blk[p, k, j*128+o] = w[ob*512 + j*128 + o, k*128 + p]
            def emit_w_chunk(wt_blk, ob, j):
                halves = load_cast(w, (ob * OJ + j) * P)

                def dest(k0, tp):
                    nc.scalar.copy(
                        out=wt_blk[:, k0:k0 + KP, j * P:(j + 1) * P],
                        in_=tp[:].rearrange("p (k b) -> p k b", k=KP),
                    )

                transpose_into(halves, dest)

            def emit_mm(wt_blk, ob, bt, out_eng=None):
                psum = mpsum_pool.tile([P, 512], f32)
                if use_double_row:
                    for q in range(KT // 2):
                        nc.tensor.matmul(
                            psum[:],
                            xt[bt][:, 2 * q:2 * q + 2, :],
                            wt_blk[:, 2 * q:2 * q + 2, :],
                            start=(q == 0),
                            stop=(q == KT // 2 - 1),
                            perf_mode=mybir.MatmulPerfMode.DoubleRow,
                        )
                else:
                    for k in range(KT):
                        nc.tensor.matmul(
                            psum[:],
                            xt[bt][:, k, :],
                            wt_blk[:, k, :],
                            start=(k == 0),
                            stop=(k == KT - 1),
                        )
                ot = out_pool.tile([P, 512], f32)
                # sign(v) for integer v: clamp to [-1, 1]
                nc.vector.tensor_scalar(
                    out=ot[:], in0=psum[:], scalar1=1.0, scalar2=-1.0,
                    op0=mybir.AluOpType.min, op1=mybir.AluOpType.max,
                )
                (out_eng or nc.gpsimd).dma_start(
                    out=out[bt * P:(bt + 1) * P, ob * 512:(ob + 1) * 512],
                    in_=ot[:],
                )

            # All of transposed W fits in SBUF at fp8 (32KB/partition), so
            # production (DMA+cast+transpose+evict) of x and W slabs is
            # spread evenly across the whole kernel, interleaved ~1:1 with
            # matmul blocks; engines stay balanced instead of front-loading
            # all x work into o-block 0.
            wt_blks = {
                ob: wt_pool.tile([P, KT, 512], mm_dt, tag="wtblk",
                                 name=f"wt{ob}")
                for ob in range(OB)
            }

            production = []
            x_left = list(range(BT))
            x_per_round = max(1, -(-BT // OB))
            for ob in range(OB):
                for bt in x_left[:x_per_round]:
                    production.append(("x", bt))
                x_left = x_left[x_per_round:]
                for j in range(OJ):
                    production.append(("w", ob, j))
            for bt in x_left:
                production.append(("x", bt))

            x_done, w_done = set(), set()
            mm_todo = [(ob, bt) for ob in range(OB) for bt in range(BT)]

            def flush_mms(limit, xs, ws, out_eng=None):
                n = 0
                for item in list(mm_todo):
                    ob, bt = item
                    if ob in ws and bt in xs and n < limit:
                        emit_mm(wt_blks[ob], ob, bt, out_eng)
                        mm_todo.remove(item)
                        n += 1

            # flush with one-production-lag availability so a matmul block
            # never waits on the eviction of the slab emitted right before it
            prev_x, prev_w = set(), set()
            n_produced = 0
            for item in production:
                # short, fully-closed dummy-matmul groups during the
                # transpose-heavy ramp: HAM ignores transpose-mode activity,
                # so without matmul traffic the PE drops to half clock until
                # real matmul blocks start flowing (~6 slabs in)
                if 0 < n_produced < 7:
                    for i in range(4):
                        nc.tensor.matmul(
                            warm_psum[:], warm_src[:, :P], warm_src[:],
                            start=(i == 0), stop=(i == 3),
                        )
                n_produced += 1
                if item[0] == "x":
                    emit_x(item[1])
                    x_done.add(item[1])
                else:
                    _, ob, j = item
                    emit_w_chunk(wt_blks[ob], ob, j)
                    if j == OJ - 1:
                        w_done.add(ob)
                flush_mms(2, prev_x, prev_w)
                prev_x, prev_w = set(x_done), set(w_done)
            flush_mms(len(mm_todo), x_done, w_done, out_eng=nc.sync)

    nc.finalize()
    return nc


def _get_nc():
    if "nc" not in _cache:
        _cache["nc"] = build_kernel()
    return _cache["nc"]


def run_sharded(input_b, weight, trace=False):
    """Run the SPMD kernel; returns (output, BassKernelResults)."""
    from concourse.bass_utils import run_bass_kernel_spmd

    nc = _get_nc()
    input_b = np.ascontiguousarray(input_b, dtype=np.float32)
    weight = np.ascontiguousarray(weight, dtype=np.float32)
    in_maps = [
        {"x": input_b[c * SHARD:(c + 1) * SHARD], "w": weight}
        for c in range(N_CORES)
    ]
    res = run_bass_kernel_spmd(nc, in_maps, list(range(N_CORES)), trace=trace)
    out = np.concatenate([res.results[c]["out"] for c in range(N_CORES)], axis=0)
    return out, res


def kernel(input_b, weight):
    out, _ = run_sharded(input_b, weight, trace=False)
    return out
